# revision 4
# baseline (speedup 1.0000x reference)
"""MDTA (Restormer transposed attention) Trainium2 Bass kernel.

Strategy: data-parallel over batch (8 cores x 1 image each). Per core:
  1. qkv 1x1 conv as f32r GEMM (PE), channel-chunked (5 M-chunks of <=128).
  2. depthwise 3x3 conv as 9 diagonal-lhsT matmuls per chunk-tile accumulated
     in PSUM (PE), with AP-clipped image-edge taps + strided DVE fixes for
     row-wrap contamination of the dx=+-1 taps.
  3. q,k: L2 norms (ACT Square+accum), scale q_hat=temp*q/||q||, k_hat=k/||k||
     in channel-part layout, PE-transpose 128x128 blocks to spatial-part
     layout, attention q_hat^T k_hat accumulated over 128 spatial blocks.
  4. block-diag mask + softmax -> A; fold output projection: M^T = A^T w_out^T
     (small PE matmuls); y = M^T.T @ v_dw as a single GEMM over v.
"""
import sys
sys.path.insert(0, "/opt/trn_rl_repo")
from contextlib import ExitStack

import numpy as np
import ml_dtypes

import concourse.bass as bass
import concourse.mybir as mybir
import concourse.tile as tile
from concourse import bacc
from concourse.bass_utils import run_bass_kernel_spmd

F32 = mybir.dt.float32
F32R = mybir.dt.float32r
BF16 = mybir.dt.bfloat16
AF = mybir.ActivationFunctionType
ALU = mybir.AluOpType
AX = mybir.AxisListType

C = 192
NHEADS = 8
HDIM = 24
H = W = 128
N = H * W            # 16384 spatial positions
NT = 512             # free-dim tile (4 image rows)
NTILES = N // NT     # 32
EPS = 1e-12
# output-channel chunks of the 576-row qkv: (start, width, role)
# roles: q = 0..191, k = 192..383, v = 384..575
CHUNKS = [(0, 128), (128, 128), (256, 128), (384, 128), (512, 64)]
TAPS = [(dy, dx) for dy in (-1, 0, 1) for dx in (-1, 0, 1)]
TAP_ORDER = [4, 0, 1, 2, 3, 5, 6, 7, 8]  # center tap first (start=True, full width)


def build_nc():
    nc = bacc.Bacc("TRN2", target_bir_lowering=False, debug=False)
    x_d = nc.dram_tensor("x", [C, N], F32R, kind="ExternalInput")
    wq_d = nc.dram_tensor("w_qkvT", [C, 576], F32R, kind="ExternalInput")
    wdd_d = nc.dram_tensor("w_dwd", [5, 128, 9 * 128], BF16, kind="ExternalInput")
    wdn_d = nc.dram_tensor("w_dwn", [5, 128, 9], F32, kind="ExternalInput")  # negated taps
    wo_d = nc.dram_tensor("w_outT", [C, C], F32, kind="ExternalInput")
    tmp_d = nc.dram_tensor("temp", [C, 1], F32, kind="ExternalInput")
    mask_d = nc.dram_tensor("mask", [C, C], F32, kind="ExternalInput")
    eye_d = nc.dram_tensor("eye", [128, 128], BF16, kind="ExternalInput")
    y_d = nc.dram_tensor("y", [C, N], F32, kind="ExternalOutput")

    with tile.TileContext(nc) as tc, ExitStack() as ctx:
        wp = ctx.enter_context(tc.tile_pool(name="w", bufs=1))
        xp = ctx.enter_context(tc.tile_pool(name="x", bufs=3))
        prep = ctx.enter_context(tc.tile_pool(name="pre", bufs=1))
        dwdp = ctx.enter_context(tc.tile_pool(name="dwd", bufs=2))
        sp = ctx.enter_context(tc.tile_pool(name="small", bufs=1))
        yp = ctx.enter_context(tc.tile_pool(name="y", bufs=3))
        gps = ctx.enter_context(tc.tile_pool(name="gps", bufs=2, space="PSUM"))
        dps = ctx.enter_context(tc.tile_pool(name="dps", bufs=2, space="PSUM"))

        # ---- persistent weights ----
        wq0 = wp.tile([128, 576], F32R, tag="wq0")
        wq1 = wp.tile([64, 576], F32R, tag="wq1")
        nc.sync.dma_start(wq0[:], wq_d[0:128, :])
        nc.sync.dma_start(wq1[:], wq_d[128:192, :])
        wo0 = wp.tile([128, C], F32, tag="wo0")
        wo1 = wp.tile([64, C], F32, tag="wo1")
        nc.sync.dma_start(wo0[:], wo_d[0:128, :])
        nc.sync.dma_start(wo1[:], wo_d[128:192, :])
        eye_s = wp.tile([128, 128], BF16, tag="eye")
        nc.sync.dma_start(eye_s[:], eye_d[:])
        tmp0 = wp.tile([128, 1], F32, tag="tmp0")
        tmp1 = wp.tile([64, 1], F32, tag="tmp1")
        nc.sync.dma_start(tmp0[:], tmp_d[0:128, :])
        nc.sync.dma_start(tmp1[:], tmp_d[128:192, :])
        mask0 = wp.tile([128, C], F32, tag="mask0")
        mask1 = wp.tile([64, C], F32, tag="mask1")
        nc.sync.dma_start(mask0[:], mask_d[0:128, :])
        nc.sync.dma_start(mask1[:], mask_d[128:192, :])
        # M^T (built later, used in phase 2)
        mt0 = wp.tile([128, C], BF16, tag="mt0")
        mt1 = wp.tile([64, C], BF16, tag="mt1")

        def gemm_chunk(ci, dst, dst_row0=0):
            """qkv GEMM for chunk ci into dst[dst_row0:dst_row0+mw, :] (bf16)."""
            mc0, mw = CHUNKS[ci]
            for t in range(NTILES):
                xt0 = xp.tile([128, NT], F32R, tag="x0")
                xt1 = xp.tile([64, NT], F32R, tag="x1")
                nc.sync.dma_start(xt0[:], x_d[0:128, t * NT:(t + 1) * NT])
                nc.sync.dma_start(xt1[:], x_d[128:192, t * NT:(t + 1) * NT])
                pg = gps.tile([mw, NT], F32, tag="g")
                nc.tensor.matmul(pg[:], wq0[:, mc0:mc0 + mw], xt0[:],
                                 start=True, stop=False)
                nc.tensor.matmul(pg[:], wq1[:, mc0:mc0 + mw], xt1[:],
                                 start=False, stop=True)
                nc.any.tensor_copy(
                    dst[dst_row0:dst_row0 + mw, t * NT:(t + 1) * NT], pg[:])

        def load_dwd(ci):
            mw = CHUNKS[ci][1]
            dwd = dwdp.tile([128, 9 * 128], BF16, tag="dwd")
            nc.sync.dma_start(dwd[:], wdd_d[ci])
            dwn = dwdp.tile([128, 9], F32, tag="dwn")
            nc.sync.dma_start(dwn[:], wdn_d[ci])
            return dwd, dwn

        def dwconv_chunk(ci, pre, dwd, dst, dst_row0=0):
            """Depthwise 3x3 of pre (mw,N) bf16 -> dst[dst_row0:...] bf16."""
            mc0, mw = CHUNKS[ci]
            for t in range(NTILES):
                base = t * NT
                pd = dps.tile([mw, NT], F32, tag="d")
                first = True
                for ti in TAP_ORDER:
                    dy, dx = TAPS[ti]
                    sh = dy * W + dx
                    lo = max(0, -(base + sh))
                    hi = min(NT, N - base - sh)
                    nc.tensor.matmul(
                        pd[:, lo:hi],
                        dwd[0:mw, ti * 128:ti * 128 + mw],
                        pre[0:mw, base + sh + lo:base + sh + hi],
                        start=first, stop=(ti == TAP_ORDER[-1]))
                    first = False
                nc.any.tensor_copy(
                    dst[dst_row0:dst_row0 + mw, base:base + NT], pd[:])

        def edge_fixes(ci, pre, dwn, dst, dst_row0=0):
            """Subtract row-wrap contamination of dx=+-1 taps (strided STT)."""
            mw = CHUNKS[ci][1]
            for dy in (-1, 0, 1):
                ti_l = (dy + 1) * 3 + 0   # (dy, dx=-1)
                y0, y1 = max(0, 1 - dy), min(127, 128 - dy)
                out_ap = dst[dst_row0:dst_row0 + mw, y0 * W:y1 * W + 1:W]
                src_ap = pre[0:mw, (y0 + dy) * W - 1:(y1 + dy) * W:W]
                nc.vector.scalar_tensor_tensor(
                    out=out_ap, in0=src_ap, scalar=dwn[0:mw, ti_l:ti_l + 1],
                    in1=out_ap, op0=ALU.mult, op1=ALU.add)
                ti_r = (dy + 1) * 3 + 2   # (dy, dx=+1)
                y0, y1 = max(0, -1 - dy), min(127, 126 - dy)
                out_ap = dst[dst_row0:dst_row0 + mw,
                             y0 * W + W - 1:y1 * W + W:W]
                src_ap = pre[0:mw, (y0 + dy + 1) * W:(y1 + dy + 1) * W + 1:W]
                nc.vector.scalar_tensor_tensor(
                    out=out_ap, in0=src_ap, scalar=dwn[0:mw, ti_r:ti_r + 1],
                    in1=out_ap, op0=ALU.mult, op1=ALU.add)

        # ================= phase 1: q,k =================
        with tc.tile_pool(name="kT", bufs=1) as kTp, \
             tc.tile_pool(name="dwout", bufs=1) as dwp, \
             tc.tile_pool(name="junk", bufs=1) as jp, \
             tc.tile_pool(name="qt", bufs=3) as qtp, \
             tc.tile_pool(name="asb", bufs=1) as ap_, \
             tc.tile_pool(name="tps", bufs=2, space="PSUM") as tps, \
             tc.tile_pool(name="aps", bufs=2, space="PSUM") as aps:

            kT = kTp.tile([128, 128 * C], BF16, tag="kT")  # block b at cols b*192
            junk = jp.tile([128, 4096], BF16, tag="junk")
            stat = sp.tile([128, 8], F32, tag="stat")
            apq0 = aps.tile([128, C], F32, tag="attn")
            apq1 = aps.tile([64, C], F32, tag="attn")

            def norms_scale(ci, dw, is_q_lo, is_k_hi, q_off=None):
                """Compute 1/max(||row||,eps) (*temp for q rows), scale dw."""
                mw = CHUNKS[ci][1]
                for s4 in range(4):
                    nc.scalar.activation(
                        junk[0:mw, :], dw[0:mw, s4 * 4096:(s4 + 1) * 4096],
                        AF.Square, accum_out=stat[0:mw, s4:s4 + 1])
                n2 = sp.tile([128, 1], F32, tag="n2")
                nc.vector.tensor_reduce(n2[0:mw, :], stat[0:mw, 0:4],
                                        axis=AX.X, op=ALU.add)
                nc.scalar.activation(n2[0:mw, :], n2[0:mw, :], AF.Sqrt)
                nc.vector.tensor_scalar_max(n2[0:mw, :], n2[0:mw, :], EPS)
                rs = sp.tile([128, 1], F32, tag="rs")
                nc.vector.reciprocal(rs[0:mw, :], n2[0:mw, :])
                if is_q_lo:  # some leading rows are q channels: multiply temp
                    qw, toff = is_q_lo
                    nc.vector.tensor_tensor(
                        rs[0:qw, :], rs[0:qw, :],
                        (tmp0 if toff < 128 else tmp1)[toff % 128:toff % 128 + qw, :],
                        op=ALU.mult)
                nc.vector.tensor_scalar_mul(dw[0:mw, :], dw[0:mw, :], rs[0:mw, :])

            def transpose_blocks(ci, dw, k_cols=None, q_rows=None):
                """PE-transpose dw blocks; k cols -> kT, q cols -> attn matmuls.

                k_cols: (part_lo, part_hi, kT_off) slice of dw partitions that
                        are k channels; q_rows: (part_lo, part_hi, attn_psum,
                        attn_row0) for q channels.
                """
                mw = CHUNKS[ci][1]
                for b in range(128):
                    pt = tps.tile([128, mw], BF16, tag="t")
                    nc.tensor.transpose(pt[:, 0:mw], dw[0:mw, b * 128:(b + 1) * 128],
                                        eye_s[0:mw, 0:mw])
                    if k_cols is not None:
                        plo, phi, koff = k_cols
                        nc.any.tensor_copy(
                            kT[:, b * C + koff:b * C + koff + (phi - plo)],
                            pt[:, plo:phi])
                    if q_rows is not None:
                        plo, phi, apsum, arow0 = q_rows
                        qw = phi - plo
                        qtb = qtp.tile([128, 128], BF16, tag="qt")
                        nc.any.tensor_copy(qtb[:, 0:qw], pt[:, plo:phi])
                        nc.tensor.matmul(
                            apsum[arow0:arow0 + qw, :], qtb[:, 0:qw],
                            kT[:, b * C:(b + 1) * C],
                            start=(b == 0), stop=(b == 127),
                            skip_group_check=True)

            # ---- chunk 2: k channels 256..383 (k-local 64..191) ----
            pre = prep.tile([128, N], BF16, tag="pre")
            dwd, dwn = load_dwd(2)
            gemm_chunk(2, pre)
            dw = dwp.tile([128, N], BF16, tag="dw")
            dwconv_chunk(2, pre, dwd, dw)
            edge_fixes(2, pre, dwn, dw)
            norms_scale(2, dw, None, None)
            transpose_blocks(2, dw, k_cols=(0, 128, 64))

            # ---- chunk 1: q 128..191 (parts 0..63) + k 192..255 (parts 64..127) ----
            pre = prep.tile([128, N], BF16, tag="pre")
            dwd, dwn = load_dwd(1)
            gemm_chunk(1, pre)
            dw = dwp.tile([128, N], BF16, tag="dw")
            dwconv_chunk(1, pre, dwd, dw)
            edge_fixes(1, pre, dwn, dw)
            norms_scale(1, dw, (64, 128), None)
            transpose_blocks(1, dw, k_cols=(64, 128, 0),
                             q_rows=(0, 64, apq1, 0))

            # ---- chunk 0: q channels 0..127 ----
            pre = prep.tile([128, N], BF16, tag="pre")
            dwd, dwn = load_dwd(0)
            gemm_chunk(0, pre)
            dw = dwp.tile([128, N], BF16, tag="dw")
            dwconv_chunk(0, pre, dwd, dw)
            edge_fixes(0, pre, dwn, dw)
            norms_scale(0, dw, (128, 0), None)
            transpose_blocks(0, dw, q_rows=(0, 128, apq0, 0))

            # ---- attention: mask + softmax + M^T ----
            def softmax_rows(apsum, msk, mw):
                a = ap_.tile([mw, C], F32, tag=f"a{mw}")
                nc.vector.tensor_tensor(a[:], apsum[:], msk[0:mw, :], op=ALU.add)
                mx = sp.tile([128, 1], F32, tag="mx")
                nc.vector.tensor_reduce(mx[0:mw, :], a[:], axis=AX.X, op=ALU.max)
                nmx = sp.tile([128, 1], F32, tag="nmx")
                nc.vector.tensor_scalar_mul(nmx[0:mw, :], mx[0:mw, :], -1.0)
                nc.scalar.activation(a[:], a[:], AF.Exp, bias=nmx[0:mw, :])
                sm = sp.tile([128, 1], F32, tag="sm")
                nc.vector.tensor_reduce(sm[0:mw, :], a[:], axis=AX.X, op=ALU.add)
                rsm = sp.tile([128, 1], F32, tag="rsm")
                nc.vector.reciprocal(rsm[0:mw, :], sm[0:mw, :])
                nc.vector.tensor_scalar_mul(a[:], a[:], rsm[0:mw, :])
                return a
            a0 = softmax_rows(apq0, mask0, 128)
            a1 = softmax_rows(apq1, mask1, 64)

            # M^T[d,o] = sum_c A[c,d] w_outT[c,o]; K = c (192 -> 2 chunks)
            for dlo, dw_, mt in ((0, 128, mt0), (128, 64, mt1)):
                pm = tps.tile([128, C], F32, tag="t")
                nc.tensor.matmul(pm[0:dw_, :], a0[:, dlo:dlo + dw_],
                                 wo0[:], start=True, stop=False)
                nc.tensor.matmul(pm[0:dw_, :], a1[:, dlo:dlo + dw_],
                                 wo1[:], start=False, stop=True)
                nc.any.tensor_copy(mt[:], pm[0:dw_, :])

        # ================= phase 2: v + output GEMM =================
        with tc.tile_pool(name="v3", bufs=1) as v3p, \
             tc.tile_pool(name="v4", bufs=1) as v4p:
            v3 = v3p.tile([128, N], BF16, tag="v3")
            v4 = v4p.tile([64, N], BF16, tag="v4")
            for ci, vt in ((3, v3), (4, v4)):
                mw = CHUNKS[ci][1]
                pre = prep.tile([mw, N], BF16, tag="pre")
                dwd, dwn = load_dwd(ci)
                gemm_chunk(ci, pre)
                dwconv_chunk(ci, pre, dwd, vt)
                edge_fixes(ci, pre, dwn, vt)

            for t in range(NTILES):
                cols = slice(t * NT, (t + 1) * NT)
                py0 = gps.tile([128, NT], F32, tag="g")
                nc.tensor.matmul(py0[:], mt0[:, 0:128], v3[:, cols],
                                 start=True, stop=False)
                nc.tensor.matmul(py0[:], mt1[:, 0:128], v4[:, cols],
                                 start=False, stop=True)
                y0 = yp.tile([128, NT], F32, tag="y0")
                nc.any.tensor_copy(y0[:], py0[:])
                nc.sync.dma_start(y_d[0:128, cols], y0[:])
                py1 = gps.tile([64, NT], F32, tag="g")
                nc.tensor.matmul(py1[:], mt0[:, 128:192], v3[:, cols],
                                 start=True, stop=False)
                nc.tensor.matmul(py1[:], mt1[:, 128:192], v4[:, cols],
                                 start=False, stop=True)
                y1 = yp.tile([64, NT], F32, tag="y1")
                nc.any.tensor_copy(y1[:], py1[:])
                nc.sync.dma_start(y_d[128:192, cols], y1[:])

    nc.compile()
    return nc


def host_inputs(x, w_qkv, w_dw, w_out, temperature):
    """Host-side prep: per-core input maps."""
    b = x.shape[0]
    w_dw9 = np.asarray(w_dw, np.float32).reshape(576, 9)
    wdd = np.zeros((5, 128, 9 * 128), np.float32)
    wdn = np.zeros((5, 128, 9), np.float32)
    for ci, (s, wid) in enumerate(CHUNKS):
        for t in range(9):
            wdd[ci, :wid, t * 128:t * 128 + wid][np.arange(wid), np.arange(wid)] = \
                w_dw9[s:s + wid, t]
        wdn[ci, :wid, :] = -w_dw9[s:s + wid, :]
    temp_pc = np.repeat(np.asarray(temperature, np.float32).reshape(NHEADS), HDIM
                        ).reshape(C, 1)
    mask = np.full((C, C), -1e9, np.float32)
    for h in range(NHEADS):
        mask[h * HDIM:(h + 1) * HDIM, h * HDIM:(h + 1) * HDIM] = 0.0
    shared = {
        "w_qkvT": np.ascontiguousarray(np.asarray(w_qkv, np.float32).T),
        "w_dwd": wdd.astype(ml_dtypes.bfloat16),
        "w_dwn": wdn,
        "w_outT": np.ascontiguousarray(np.asarray(w_out, np.float32).T),
        "temp": temp_pc,
        "mask": mask,
        "eye": np.eye(128, dtype=ml_dtypes.bfloat16),
    }
    return [dict(shared, x=np.ascontiguousarray(
        np.asarray(x[c], np.float32).reshape(C, N))) for c in range(b)]


_NC_CACHE = {}


def kernel(x, w_qkv, w_dw, w_out, temperature):
    x = np.asarray(x)
    if "nc" not in _NC_CACHE:
        _NC_CACHE["nc"] = build_nc()
    nc = _NC_CACHE["nc"]
    in_maps = host_inputs(x, w_qkv, w_dw, w_out, temperature)
    res = run_bass_kernel_spmd(nc, in_maps, list(range(8)))
    out = np.stack([res.results[c]["y"].reshape(C, H, W) for c in range(8)])
    return out.astype(np.float32)


# revision 20
# speedup vs baseline: 5.8955x; 5.8955x over previous
"""MDTA (Restormer transposed attention) Trainium2 Bass kernel.

Strategy: data-parallel over batch (8 cores x 1 image each). Per core:
  1. qkv 1x1 conv as f32r GEMM (PE), channel-chunked (5 M-chunks of <=128).
  2. depthwise 3x3 conv as 9 diagonal-lhsT matmuls per chunk-tile accumulated
     in PSUM (PE), with AP-clipped image-edge taps + strided DVE fixes for
     row-wrap contamination of the dx=+-1 taps.
  3. q,k: L2 norms (ACT Square+accum), scale q_hat=temp*q/||q||, k_hat=k/||k||
     in channel-part layout, PE-transpose 128x128 blocks to spatial-part
     layout, attention q_hat^T k_hat accumulated over 128 spatial blocks.
  4. block-diag mask + softmax -> A; fold output projection: M^T = A^T w_out^T
     (small PE matmuls); y = M^T.T @ v_dw as a single GEMM over v.
"""
import sys
sys.path.insert(0, "/opt/trn_rl_repo")
from contextlib import ExitStack

import numpy as np
import ml_dtypes

import concourse.bass as bass
import concourse.mybir as mybir
import concourse.tile as tile
from concourse import bacc
from concourse.bass_utils import run_bass_kernel_spmd

F32 = mybir.dt.float32
F32R = mybir.dt.float32r
BF16 = mybir.dt.bfloat16
AF = mybir.ActivationFunctionType
ALU = mybir.AluOpType
AX = mybir.AxisListType

C = 192
NHEADS = 8
HDIM = 24
H = W = 128
N = H * W            # 16384 spatial positions
NT = 512             # free-dim tile (4 image rows)
NTILES = N // NT     # 32
EPS = 1e-12
# output-channel chunks of the 576-row qkv: (start, width, role)
# roles: q = 0..191, k = 192..383, v = 384..575
CHUNKS = [(0, 128), (128, 128), (256, 128), (384, 128), (512, 64)]
TAPS = [(dy, dx) for dy in (-1, 0, 1) for dx in (-1, 0, 1)]
TAP_ORDER = [4, 0, 1, 2, 3, 5, 6, 7, 8]  # center tap first (start=True, full width)


def build_nc(reps=1, abl=()):  # noqa: C901
    nc = bacc.Bacc("TRN2", target_bir_lowering=False, debug=False)
    x_d = (nc.dram_tensor("x_scratch", [C, N], F32) if "dummyx" in abl
           else nc.dram_tensor("x", [C, N], F32, kind="ExternalInput"))
    wq_d = nc.dram_tensor("w_qkvT", [C, 576], BF16, kind="ExternalInput")
    wdd_d = nc.dram_tensor("w_dwd", [5, 128, 9 * 128], BF16, kind="ExternalInput")
    wdn_d = nc.dram_tensor("w_dwn", [5, 128, 9], F32, kind="ExternalInput")  # negated taps
    wdp_d = nc.dram_tensor("w_dwp", [5, 128, 9], F32, kind="ExternalInput")  # taps
    wo_d = nc.dram_tensor("w_outT", [C, C], F32, kind="ExternalInput")
    tmp_d = nc.dram_tensor("temp", [C, 1], F32, kind="ExternalInput")
    mask_d = nc.dram_tensor("mask", [C, C], F32, kind="ExternalInput")
    eye_d = nc.dram_tensor("eye", [128, 128], BF16, kind="ExternalInput")
    if "dummyy" in abl:
        y_d = nc.dram_tensor("y_scratch", [C, N], F32)
        yprobe_d = nc.dram_tensor("y", [128, 4], F32, kind="ExternalOutput")
    else:
        y_d = nc.dram_tensor("y", [C, N], F32, kind="ExternalOutput")
        yprobe_d = None

    with tile.TileContext(nc) as tc, ExitStack() as ctx:
        wp = ctx.enter_context(tc.tile_pool(name="w", bufs=1))
        prep = ctx.enter_context(tc.tile_pool(name="pre", bufs=1))
        dwdp = ctx.enter_context(tc.tile_pool(name="dwd", bufs=1))
        sp = ctx.enter_context(tc.tile_pool(name="small", bufs=1))
        yp = ctx.enter_context(tc.tile_pool(name="y", bufs=2))
        gps = ctx.enter_context(tc.tile_pool(name="gps", bufs=2, space="PSUM"))
        dps = ctx.enter_context(tc.tile_pool(name="dps", bufs=2, space="PSUM"))

        # ---- persistent weights ----
        wq0 = wp.tile([128, 576], BF16, tag="wq0")
        wq1 = wp.tile([128, 576], BF16, tag="wq1")  # ch 128..191 duplicated
        nc.sync.dma_start(wq0[:], wq_d[0:128, :])
        nc.sync.dma_start(wq1[0:64, :], wq_d[128:192, :])
        nc.sync.dma_start(wq1[64:128, :], wq_d[128:192, :])
        wo0 = wp.tile([128, C], F32, tag="wo0")
        wo1 = wp.tile([64, C], F32, tag="wo1")
        nc.sync.dma_start(wo0[:], wo_d[0:128, :])
        nc.sync.dma_start(wo1[:], wo_d[128:192, :])
        eye_s = wp.tile([128, 128], BF16, tag="eye")
        nc.sync.dma_start(eye_s[:], eye_d[:])
        tmp0 = wp.tile([128, 1], F32, tag="tmp0")
        tmp1 = wp.tile([64, 1], F32, tag="tmp1")
        nc.sync.dma_start(tmp0[:], tmp_d[0:128, :])
        nc.sync.dma_start(tmp1[:], tmp_d[128:192, :])
        mask0 = wp.tile([128, C], F32, tag="mask0")
        mask1 = wp.tile([64, C], F32, tag="mask1")
        nc.sync.dma_start(mask0[:], mask_d[0:128, :])
        nc.sync.dma_start(mask1[:], mask_d[128:192, :])
        # M^T (built later, used in phase 2)
        mt0 = wp.tile([128, C], BF16, tag="mt0")
        mt1 = wp.tile([64, C], BF16, tag="mt1")

        # resident bf16 copy of x (loaded once; GEMM reads SBUF, PE never
        # stalls on HBM)
        xres0 = wp.tile([128, N], BF16, tag="xres0")
        xres1 = wp.tile([128, N], BF16, tag="xres1")  # ch 128..191 duplicated on parts 64..127
        with tc.tile_pool(name="xload", bufs=3) as xp:
            for t in range(NTILES):
                cols = slice(t * NT, (t + 1) * NT)
                xt0 = xp.tile([128, NT], F32, tag="x0")
                xt1 = xp.tile([64, NT], F32, tag="x1")
                nc.sync.dma_start(xt0[:], x_d[0:128, cols])
                nc.sync.dma_start(xt1[:], x_d[128:192, cols])
                nc.vector.tensor_copy(xres0[:, cols], xt0[:])
                nc.vector.tensor_copy(xres1[0:64, cols], xt1[:])
                nc.vector.tensor_copy(xres1[64:128, cols], xt1[:])

        def gemm_chunk(ci, dst, dst_row0=0):
            """qkv GEMM for chunk ci into dst[dst_row0:dst_row0+mw, :] (bf16)."""
            mc0, mw = CHUNKS[ci]
            for tp in range(NTILES // 2):
                pgs = []
                for half in (0, 1):
                    t = 2 * tp + half
                    cols = slice(t * NT, (t + 1) * NT)
                    pg = gps.tile([mw, NT], F32, tag="g")
                    pgs.append((t, cols, pg))
                    nc.tensor.matmul(pg[:], wq0[:, mc0:mc0 + mw],
                                     xres0[:, cols], start=True, stop=False)
                # the two K=64 tails run concurrently on disjoint row strips
                for half in (0, 1):
                    t, cols, pg = pgs[half]
                    p0 = 64 * half
                    nc.tensor.matmul(pg[:],
                                     wq1[p0:p0 + 64, mc0:mc0 + mw],
                                     xres1[p0:p0 + 64, cols],
                                     start=False, stop=True,
                                     tile_position=(p0, 0))
                for t, cols, pg in pgs:
                    nc.vector.tensor_copy(
                        dst[dst_row0:dst_row0 + mw, cols], pg[:])

        def load_dwd(ci):
            mw = CHUNKS[ci][1]
            dwd = dwdp.tile([128, 9 * 128], BF16, tag="dwd")
            nc.sync.dma_start(dwd[:], wdd_d[ci])
            dwn = dwdp.tile([128, 9], F32, tag="dwn")
            nc.sync.dma_start(dwn[:], wdn_d[ci])
            dwpos = dwdp.tile([128, 9], F32, tag="dwpos")
            nc.sync.dma_start(dwpos[:], wdp_d[ci])
            return dwd, dwn, dwpos

        def dwconv_chunk(ci, pre, dwd, dst, dst_row0=0, engine="pe", dwp_=None):
            """Depthwise 3x3 of pre (mw,N) bf16 -> dst[dst_row0:...] bf16."""
            mc0, mw = CHUNKS[ci]
            for t in range(NTILES):
                base = t * NT
                if engine == "pe":
                    pd = dps.tile([mw, NT], F32, tag="d")
                    first = True
                    for ti in (TAP_ORDER[:1] if "tap1" in abl else TAP_ORDER):
                        dy, dx = TAPS[ti]
                        sh = dy * W + dx
                        lo = max(0, -(base + sh))
                        hi = min(NT, N - base - sh)
                        nc.tensor.matmul(
                            pd[:, lo:hi],
                            dwd[0:mw, ti * 128:ti * 128 + mw],
                            pre[0:mw, base + sh + lo:base + sh + hi],
                            start=first, stop=(first if "tap1" in abl else (ti == TAP_ORDER[-1])))
                        first = False
                    nc.scalar.copy(
                        dst[dst_row0:dst_row0 + mw, base:base + NT], pd[:])
                else:
                    # DVE: 9 scalar_tensor_tensor FMAs straight into dst (bf16)
                    dcols = slice(dst_row0, dst_row0 + mw)
                    out = dst[dcols, base:base + NT]
                    first = True
                    for ti in (TAP_ORDER[:1] if "tap1" in abl else TAP_ORDER):
                        dy, dx = TAPS[ti]
                        sh = dy * W + dx
                        lo = max(0, -(base + sh))
                        hi = min(NT, N - base - sh)
                        src = pre[0:mw, base + sh + lo:base + sh + hi]
                        wcol = dwp_[0:mw, ti:ti + 1]
                        if first:
                            nc.vector.tensor_scalar_mul(out, src, wcol)
                            first = False
                        else:
                            nc.vector.scalar_tensor_tensor(
                                out=dst[dcols, base + lo:base + hi], in0=src,
                                scalar=wcol,
                                in1=dst[dcols, base + lo:base + hi],
                                op0=ALU.mult, op1=ALU.add)

        def edge_fixes(ci, pre, dwn, dst, dst_row0=0):
            """Subtract row-wrap contamination of dx=+-1 taps (strided STT)."""
            mw = CHUNKS[ci][1]
            for dy in (-1, 0, 1):
                ti_l = (dy + 1) * 3 + 0   # (dy, dx=-1)
                y0, y1 = max(0, 1 - dy), min(127, 128 - dy)
                out_ap = dst[dst_row0:dst_row0 + mw, y0 * W:y1 * W + 1:W]
                src_ap = pre[0:mw, (y0 + dy) * W - 1:(y1 + dy) * W:W]
                nc.vector.scalar_tensor_tensor(
                    out=out_ap, in0=src_ap, scalar=dwn[0:mw, ti_l:ti_l + 1],
                    in1=out_ap, op0=ALU.mult, op1=ALU.add)
                ti_r = (dy + 1) * 3 + 2   # (dy, dx=+1)
                y0, y1 = max(0, -1 - dy), min(127, 126 - dy)
                out_ap = dst[dst_row0:dst_row0 + mw,
                             y0 * W + W - 1:y1 * W + W:W]
                src_ap = pre[0:mw, (y0 + dy + 1) * W:(y1 + dy + 1) * W + 1:W]
                nc.vector.scalar_tensor_tensor(
                    out=out_ap, in0=src_ap, scalar=dwn[0:mw, ti_r:ti_r + 1],
                    in1=out_ap, op0=ALU.mult, op1=ALU.add)

        # ================= phase 1: q,k =================
        for _rep in range(reps):
         with tc.tile_pool(name="kT", bufs=1) as kTp, \
             tc.tile_pool(name="dwout", bufs=1) as dwp, \
             tc.tile_pool(name="junk", bufs=1) as jp, \
             tc.tile_pool(name="qt", bufs=3) as qtp, \
             tc.tile_pool(name="asb", bufs=1) as ap_, \
             tc.tile_pool(name="tps", bufs=2, space="PSUM") as tps, \
             tc.tile_pool(name="aps", bufs=2, space="PSUM") as aps:

            kT = kTp.tile([128, 128 * C], BF16, tag="kT")  # block b at cols b*192
            junk = jp.tile([128, 2048], BF16, tag="junk")
            stat = sp.tile([128, 8], F32, tag="stat")
            apq0 = aps.tile([128, C], F32, tag="attn")
            apq1 = aps.tile([64, C], F32, tag="attn")

            def norms_scale(ci, dw, is_q_lo, is_k_hi, q_off=None):
                """Compute 1/max(||row||,eps) (*temp for q rows), scale dw."""
                mw = CHUNKS[ci][1]
                for s4 in range(8):
                    nc.scalar.activation(
                        junk[0:mw, :], dw[0:mw, s4 * 2048:(s4 + 1) * 2048],
                        AF.Square, accum_out=stat[0:mw, s4:s4 + 1])
                n2 = sp.tile([128, 1], F32, tag="n2")
                nc.vector.tensor_reduce(n2[0:mw, :], stat[0:mw, 0:8],
                                        axis=AX.X, op=ALU.add)
                nc.scalar.activation(n2[0:mw, :], n2[0:mw, :], AF.Sqrt)
                nc.vector.tensor_scalar_max(n2[0:mw, :], n2[0:mw, :], EPS)
                rs = sp.tile([128, 1], F32, tag="rs")
                nc.vector.reciprocal(rs[0:mw, :], n2[0:mw, :])
                if is_q_lo:  # some leading rows are q channels: multiply temp
                    qw, toff = is_q_lo
                    nc.vector.tensor_tensor(
                        rs[0:qw, :], rs[0:qw, :],
                        (tmp0 if toff < 128 else tmp1)[toff % 128:toff % 128 + qw, :],
                        op=ALU.mult)
                nc.vector.tensor_scalar_mul(dw[0:mw, :], dw[0:mw, :], rs[0:mw, :])

            def transpose_blocks(ci, dw, k_cols=None, q_rows=None):
                """PE-transpose dw blocks; k cols -> kT, q cols -> attn matmuls.

                k_cols: (part_lo, part_hi, kT_off) slice of dw partitions that
                        are k channels; q_rows: (part_lo, part_hi, attn_psum,
                        attn_row0) for q channels.
                """
                mw = CHUNKS[ci][1]
                nblk = 1 if "notr" in abl else 128
                kT3 = kT[:].rearrange("p (blk c) -> p blk c", c=C)
                for b0 in range(0, nblk, 2):
                    npair = min(2, nblk - b0)
                    pt = tps.tile([128, 2 * mw], BF16, tag="t")
                    pt3 = pt[:].rearrange("p (two c) -> p two c", c=mw)
                    for h in range(npair):
                        b = b0 + h
                        nc.tensor.transpose(
                            pt[:, h * mw:(h + 1) * mw],
                            dw[0:mw, b * 128:(b + 1) * 128], eye_s[0:mw, 0:mw])
                    if k_cols is not None:
                        plo, phi, koff = k_cols
                        nc.scalar.copy(
                            kT3[:, b0:b0 + npair, koff:koff + (phi - plo)],
                            pt3[:, 0:npair, plo:phi])
                    if q_rows is not None:
                        plo, phi, apsum, arow0 = q_rows
                        qw = phi - plo
                        qtb = qtp.tile([128, 256], BF16, tag="qt")
                        nc.scalar.copy(qtb[:].rearrange(
                            "p (two c) -> p two c", c=128)[:, 0:npair, 0:qw],
                            pt3[:, 0:npair, plo:phi])
                        for h in range(npair):
                            b = b0 + h
                            nc.tensor.matmul(
                                apsum[arow0:arow0 + qw, :],
                                qtb[:, h * 128:h * 128 + qw],
                                kT[:, b * C:(b + 1) * C],
                                start=(b == 0),
                                stop=(b == (0 if "notr" in abl else 127)),
                                skip_group_check=True)

            # ---- chunk 2: k channels 256..383 (k-local 64..191) ----
            pre = prep.tile([128, N], BF16, tag="pre")
            dwd, dwn, dwpos = load_dwd(2)
            gemm_chunk(2, pre)
            dw = dwp.tile([128, N], BF16, tag="dw")
            dwconv_chunk(2, pre, dwd, dw)
            edge_fixes(2, pre, dwn, dw)
            norms_scale(2, dw, None, None)
            transpose_blocks(2, dw, k_cols=(0, 128, 64))

            # ---- chunk 1: q 128..191 (parts 0..63) + k 192..255 (parts 64..127) ----
            pre = prep.tile([128, N], BF16, tag="pre")
            dwd, dwn, dwpos = load_dwd(1)
            gemm_chunk(1, pre)
            dw = dwp.tile([128, N], BF16, tag="dw")
            dwconv_chunk(1, pre, dwd, dw)
            edge_fixes(1, pre, dwn, dw)
            norms_scale(1, dw, (64, 128), None)
            transpose_blocks(1, dw, k_cols=(64, 128, 0),
                             q_rows=(0, 64, apq1, 0))

            # ---- chunk 0: q channels 0..127 ----
            pre = prep.tile([128, N], BF16, tag="pre")
            dwd, dwn, dwpos = load_dwd(0)
            gemm_chunk(0, pre)
            dw = dwp.tile([128, N], BF16, tag="dw")
            dwconv_chunk(0, pre, dwd, dw)
            edge_fixes(0, pre, dwn, dw)
            norms_scale(0, dw, (128, 0), None)
            transpose_blocks(0, dw, q_rows=(0, 128, apq0, 0))

            # ---- attention: mask + softmax + M^T ----
            def softmax_rows(apsum, msk, mw):
                a = ap_.tile([mw, C], F32, tag=f"a{mw}")
                nc.vector.tensor_tensor(a[:], apsum[:], msk[0:mw, :], op=ALU.add)
                mx = sp.tile([128, 1], F32, tag="mx")
                nc.vector.tensor_reduce(mx[0:mw, :], a[:], axis=AX.X, op=ALU.max)
                nmx = sp.tile([128, 1], F32, tag="nmx")
                nc.vector.tensor_scalar_mul(nmx[0:mw, :], mx[0:mw, :], -1.0)
                nc.scalar.activation(a[:], a[:], AF.Exp, bias=nmx[0:mw, :])
                sm = sp.tile([128, 1], F32, tag="sm")
                nc.vector.tensor_reduce(sm[0:mw, :], a[:], axis=AX.X, op=ALU.add)
                rsm = sp.tile([128, 1], F32, tag="rsm")
                nc.vector.reciprocal(rsm[0:mw, :], sm[0:mw, :])
                nc.vector.tensor_scalar_mul(a[:], a[:], rsm[0:mw, :])
                return a
            a0 = softmax_rows(apq0, mask0, 128)
            a1 = softmax_rows(apq1, mask1, 64)

            # M^T[d,o] = sum_c A[c,d] w_outT[c,o]; K = c (192 -> 2 chunks)
            for dlo, dw_, mt in ((0, 128, mt0), (128, 64, mt1)):
                pm = tps.tile([128, C], F32, tag="t")
                nc.tensor.matmul(pm[0:dw_, :], a0[:, dlo:dlo + dw_],
                                 wo0[:], start=True, stop=False)
                nc.tensor.matmul(pm[0:dw_, :], a1[:, dlo:dlo + dw_],
                                 wo1[:], start=False, stop=True)
                nc.any.tensor_copy(mt[:], pm[0:dw_, :])

         with tc.tile_pool(name="v3", bufs=1) as v3p, \
             tc.tile_pool(name="v4", bufs=1) as v4p:
            v3 = v3p.tile([128, N], BF16, tag="v3")
            v4 = v4p.tile([64, N], BF16, tag="v4")
            for ci, vt in ((3, v3), (4, v4)):
                mw = CHUNKS[ci][1]
                pre = prep.tile([mw, N], BF16, tag="pre")
                dwd, dwn, dwpos = load_dwd(ci)
                gemm_chunk(ci, pre)
                dwconv_chunk(ci, pre, dwd, vt)
                edge_fixes(ci, pre, dwn, vt)

            for t in range(NTILES):
                cols = slice(t * NT, (t + 1) * NT)
                py0 = gps.tile([128, NT], F32, tag="g")
                nc.tensor.matmul(py0[:], mt0[:, 0:128], v3[:, cols],
                                 start=True, stop=False)
                nc.tensor.matmul(py0[:], mt1[:, 0:128], v4[:, cols],
                                 start=False, stop=True)
                y0 = yp.tile([128, NT], F32, tag="y0")
                nc.any.tensor_copy(y0[:], py0[:])
                nc.sync.dma_start(y_d[0:128, cols], y0[:])
                py1 = gps.tile([64, NT], F32, tag="g")
                nc.tensor.matmul(py1[:], mt0[:, 128:192], v3[:, cols],
                                 start=True, stop=False)
                nc.tensor.matmul(py1[:], mt1[:, 128:192], v4[:, cols],
                                 start=False, stop=True)
                y1 = yp.tile([64, NT], F32, tag="y1")
                nc.any.tensor_copy(y1[:], py1[:])
                nc.sync.dma_start(y_d[128:192, cols], y1[:])
            if yprobe_d is not None:
                nc.sync.dma_start(yprobe_d[:], y0[:, 0:4])

    nc.compile()
    return nc


def host_inputs(x, w_qkv, w_dw, w_out, temperature):
    """Host-side prep: per-core input maps."""
    b = x.shape[0]
    w_dw9 = np.asarray(w_dw, np.float32).reshape(576, 9)
    wdd = np.zeros((5, 128, 9 * 128), np.float32)
    wdn = np.zeros((5, 128, 9), np.float32)
    for ci, (s, wid) in enumerate(CHUNKS):
        for t in range(9):
            wdd[ci, :wid, t * 128:t * 128 + wid][np.arange(wid), np.arange(wid)] = \
                w_dw9[s:s + wid, t]
        wdn[ci, :wid, :] = -w_dw9[s:s + wid, :]
    temp_pc = np.repeat(np.asarray(temperature, np.float32).reshape(NHEADS), HDIM
                        ).reshape(C, 1)
    mask = np.full((C, C), -1e9, np.float32)
    for h in range(NHEADS):
        mask[h * HDIM:(h + 1) * HDIM, h * HDIM:(h + 1) * HDIM] = 0.0
    shared = {
        "w_qkvT": np.ascontiguousarray(np.asarray(w_qkv, np.float32).T
                                       ).astype(ml_dtypes.bfloat16),
        "w_dwd": wdd.astype(ml_dtypes.bfloat16),
        "w_dwn": wdn,
        "w_dwp": -wdn,
        "w_outT": np.ascontiguousarray(np.asarray(w_out, np.float32).T),
        "temp": temp_pc,
        "mask": mask,
        "eye": np.eye(128, dtype=ml_dtypes.bfloat16),
    }
    return [dict(shared, x=np.ascontiguousarray(
        np.asarray(x[c], np.float32).reshape(C, N))) for c in range(b)]


_NC_CACHE = {}


def kernel(x, w_qkv, w_dw, w_out, temperature):
    x = np.asarray(x)
    if "nc" not in _NC_CACHE:
        _NC_CACHE["nc"] = build_nc()
    nc = _NC_CACHE["nc"]
    in_maps = host_inputs(x, w_qkv, w_dw, w_out, temperature)
    res = run_bass_kernel_spmd(nc, in_maps, list(range(8)))
    out = np.stack([res.results[c]["y"].reshape(C, H, W) for c in range(8)])
    return out.astype(np.float32)


# revision 21
# speedup vs baseline: 26.7578x; 4.5387x over previous
"""MDTA (Restormer transposed attention) Trainium2 Bass kernel.

Strategy: data-parallel over batch (8 cores x 1 image each). Per core:
  1. qkv 1x1 conv as bf16 GEMM (PE) from an SBUF-resident bf16 copy of x,
     channel-chunked (5 M-chunks of <=128), K=64 tails row-packed in pairs.
  2. depthwise 3x3 conv as 9 diagonal-lhsT matmuls per chunk-tile accumulated
     in PSUM (PE), with AP-clipped image-edge taps + strided DVE fixes for
     row-wrap contamination of the dx=+-1 taps.
  3. q,k: L2 norms (ACT Square+accum), scale q_hat=temp*q/||q||, k_hat=k/||k||
     in channel-part layout, PE-transpose 128x128 blocks to spatial-part
     layout, attention q_hat^T k_hat accumulated over 128 spatial blocks.
  4. block-diag mask + softmax -> A; fold output projection: M^T = A^T w_out^T
     (small PE matmuls); y = M^T.T @ v_dw as a single GEMM over v.
"""
import sys
sys.path.insert(0, "/opt/trn_rl_repo")
from contextlib import ExitStack

import numpy as np
import ml_dtypes

import concourse.bass as bass
import concourse.mybir as mybir
import concourse.tile as tile
from concourse import bacc
from concourse.bass_utils import run_bass_kernel_spmd

F32 = mybir.dt.float32
F32R = mybir.dt.float32r
BF16 = mybir.dt.bfloat16
AF = mybir.ActivationFunctionType
ALU = mybir.AluOpType
AX = mybir.AxisListType

C = 192
NHEADS = 8
HDIM = 24
H = W = 128
N = H * W            # 16384 spatial positions
NT = 512             # free-dim tile (4 image rows)
NTILES = N // NT     # 32
EPS = 1e-12
# output-channel chunks of the 576-row qkv: (start, width, role)
# roles: q = 0..191, k = 192..383, v = 384..575
CHUNKS = [(0, 128), (128, 128), (256, 128), (384, 128), (512, 64)]
TAPS = [(dy, dx) for dy in (-1, 0, 1) for dx in (-1, 0, 1)]
TAP_ORDER = [4, 0, 1, 2, 3, 5, 6, 7, 8]  # center tap first (start=True, full width)


def build_nc(reps=1, abl=()):  # noqa: C901
    nc = bacc.Bacc("TRN2", target_bir_lowering=False, debug=False)
    x_d = (nc.dram_tensor("x_scratch", [C, N], F32) if "dummyx" in abl
           else nc.dram_tensor("x", [C, N], F32, kind="ExternalInput"))
    wq_d = nc.dram_tensor("w_qkvT", [C, 576], BF16, kind="ExternalInput")
    wdd_d = nc.dram_tensor("w_dwd", [5, 128, 9 * 128], BF16, kind="ExternalInput")
    wdn_d = nc.dram_tensor("w_dwn", [5, 128, 9], F32, kind="ExternalInput")  # negated taps
    wdp_d = nc.dram_tensor("w_dwp", [5, 128, 9], F32, kind="ExternalInput")  # taps
    wo_d = nc.dram_tensor("w_outT", [C, C], F32, kind="ExternalInput")
    tmp_d = nc.dram_tensor("temp", [C, 1], F32, kind="ExternalInput")
    mask_d = nc.dram_tensor("mask", [C, C], F32, kind="ExternalInput")
    eye_d = nc.dram_tensor("eye", [128, 128], BF16, kind="ExternalInput")
    if "dummyy" in abl:
        y_d = nc.dram_tensor("y_scratch", [C, N], F32)
        yprobe_d = nc.dram_tensor("y", [128, 4], F32, kind="ExternalOutput")
    else:
        y_d = nc.dram_tensor("y", [C, N], F32, kind="ExternalOutput")
        yprobe_d = None

    with tile.TileContext(nc) as tc, ExitStack() as ctx:
        wp = ctx.enter_context(tc.tile_pool(name="w", bufs=1))
        prep = ctx.enter_context(tc.tile_pool(name="pre", bufs=1))
        dwdp = ctx.enter_context(tc.tile_pool(name="dwd", bufs=1))
        sp = ctx.enter_context(tc.tile_pool(name="small", bufs=1))
        yp = ctx.enter_context(tc.tile_pool(name="y", bufs=2))
        gps = ctx.enter_context(tc.tile_pool(name="gps", bufs=2, space="PSUM"))
        dps = ctx.enter_context(tc.tile_pool(name="dps", bufs=2, space="PSUM"))

        # ---- persistent weights ----
        wq0 = wp.tile([128, 576], BF16, tag="wq0")
        wq1 = wp.tile([128, 576], BF16, tag="wq1")  # ch 128..191 duplicated
        nc.sync.dma_start(wq0[:], wq_d[0:128, :])
        nc.sync.dma_start(wq1[0:64, :], wq_d[128:192, :])
        nc.sync.dma_start(wq1[64:128, :], wq_d[128:192, :])
        wo0 = wp.tile([128, C], F32, tag="wo0")
        wo1 = wp.tile([64, C], F32, tag="wo1")
        nc.sync.dma_start(wo0[:], wo_d[0:128, :])
        nc.sync.dma_start(wo1[:], wo_d[128:192, :])
        eye_s = wp.tile([128, 128], BF16, tag="eye")
        nc.sync.dma_start(eye_s[:], eye_d[:])
        tmp0 = wp.tile([128, 1], F32, tag="tmp0")
        tmp1 = wp.tile([64, 1], F32, tag="tmp1")
        nc.sync.dma_start(tmp0[:], tmp_d[0:128, :])
        nc.sync.dma_start(tmp1[:], tmp_d[128:192, :])
        mask0 = wp.tile([128, C], F32, tag="mask0")
        mask1 = wp.tile([64, C], F32, tag="mask1")
        nc.sync.dma_start(mask0[:], mask_d[0:128, :])
        nc.sync.dma_start(mask1[:], mask_d[128:192, :])
        # M^T (built later, used in phase 2)
        mt0 = wp.tile([128, C], BF16, tag="mt0")
        mt1 = wp.tile([64, C], BF16, tag="mt1")

        # resident bf16 copy of x (loaded once; GEMM reads SBUF, PE never
        # stalls on HBM)
        xres0 = wp.tile([128, N], BF16, tag="xres0")
        xres1 = wp.tile([128, N], BF16, tag="xres1")  # ch 128..191 duplicated on parts 64..127
        with tc.tile_pool(name="xload", bufs=3) as xp:
            for t in range(NTILES):
                cols = slice(t * NT, (t + 1) * NT)
                xt0 = xp.tile([128, NT], F32, tag="x0")
                xt1 = xp.tile([64, NT], F32, tag="x1")
                nc.sync.dma_start(xt0[:], x_d[0:128, cols])
                nc.sync.dma_start(xt1[:], x_d[128:192, cols])
                nc.vector.tensor_copy(xres0[:, cols], xt0[:])
                nc.vector.tensor_copy(xres1[0:64, cols], xt1[:])
                nc.vector.tensor_copy(xres1[64:128, cols], xt1[:])

        def gemm_chunk(ci, dst, dst_row0=0):
            """qkv GEMM for chunk ci into dst[dst_row0:dst_row0+mw, :] (bf16)."""
            mc0, mw = CHUNKS[ci]
            for tp in range(NTILES // 2):
                pgs = []
                for half in (0, 1):
                    t = 2 * tp + half
                    cols = slice(t * NT, (t + 1) * NT)
                    pg = gps.tile([mw, NT], F32, tag="g")
                    pgs.append((t, cols, pg))
                    nc.tensor.matmul(pg[:], wq0[:, mc0:mc0 + mw],
                                     xres0[:, cols], start=True, stop=False)
                # the two K=64 tails run concurrently on disjoint row strips
                for half in (0, 1):
                    t, cols, pg = pgs[half]
                    p0 = 64 * half
                    nc.tensor.matmul(pg[:],
                                     wq1[p0:p0 + 64, mc0:mc0 + mw],
                                     xres1[p0:p0 + 64, cols],
                                     start=False, stop=True,
                                     tile_position=(p0, 0))
                for t, cols, pg in pgs:
                    nc.vector.tensor_copy(
                        dst[dst_row0:dst_row0 + mw, cols], pg[:])

        def load_dwd(ci):
            mw = CHUNKS[ci][1]
            dwd = dwdp.tile([128, 9 * 128], BF16, tag="dwd")
            nc.sync.dma_start(dwd[:], wdd_d[ci])
            dwn = dwdp.tile([128, 9], F32, tag="dwn")
            nc.sync.dma_start(dwn[:], wdn_d[ci])
            dwpos = dwdp.tile([128, 9], F32, tag="dwpos")
            nc.sync.dma_start(dwpos[:], wdp_d[ci])
            return dwd, dwn, dwpos

        def dwconv_chunk(ci, pre, dwd, dst, dst_row0=0, engine="pe", dwp_=None):
            """Depthwise 3x3 of pre (mw,N) bf16 -> dst[dst_row0:...] bf16."""
            mc0, mw = CHUNKS[ci]
            for t in range(NTILES):
                base = t * NT
                if engine == "pe":
                    pd = dps.tile([mw, NT], F32, tag="d")
                    first = True
                    for ti in (TAP_ORDER[:1] if "tap1" in abl else TAP_ORDER):
                        dy, dx = TAPS[ti]
                        sh = dy * W + dx
                        lo = max(0, -(base + sh))
                        hi = min(NT, N - base - sh)
                        nc.tensor.matmul(
                            pd[:, lo:hi],
                            dwd[0:mw, ti * 128:ti * 128 + mw],
                            pre[0:mw, base + sh + lo:base + sh + hi],
                            start=first, stop=(first if "tap1" in abl else (ti == TAP_ORDER[-1])))
                        first = False
                    nc.scalar.copy(
                        dst[dst_row0:dst_row0 + mw, base:base + NT], pd[:])
                else:
                    # DVE: 9 scalar_tensor_tensor FMAs straight into dst (bf16)
                    dcols = slice(dst_row0, dst_row0 + mw)
                    out = dst[dcols, base:base + NT]
                    first = True
                    for ti in (TAP_ORDER[:1] if "tap1" in abl else TAP_ORDER):
                        dy, dx = TAPS[ti]
                        sh = dy * W + dx
                        lo = max(0, -(base + sh))
                        hi = min(NT, N - base - sh)
                        src = pre[0:mw, base + sh + lo:base + sh + hi]
                        wcol = dwp_[0:mw, ti:ti + 1]
                        if first:
                            nc.vector.tensor_scalar_mul(out, src, wcol)
                            first = False
                        else:
                            nc.vector.scalar_tensor_tensor(
                                out=dst[dcols, base + lo:base + hi], in0=src,
                                scalar=wcol,
                                in1=dst[dcols, base + lo:base + hi],
                                op0=ALU.mult, op1=ALU.add)

        def edge_fixes(ci, pre, dwn, dst, dst_row0=0):
            """Subtract row-wrap contamination of dx=+-1 taps (strided STT)."""
            mw = CHUNKS[ci][1]
            for dy in (-1, 0, 1):
                ti_l = (dy + 1) * 3 + 0   # (dy, dx=-1)
                y0, y1 = max(0, 1 - dy), min(127, 128 - dy)
                out_ap = dst[dst_row0:dst_row0 + mw, y0 * W:y1 * W + 1:W]
                src_ap = pre[0:mw, (y0 + dy) * W - 1:(y1 + dy) * W:W]
                nc.vector.scalar_tensor_tensor(
                    out=out_ap, in0=src_ap, scalar=dwn[0:mw, ti_l:ti_l + 1],
                    in1=out_ap, op0=ALU.mult, op1=ALU.add)
                ti_r = (dy + 1) * 3 + 2   # (dy, dx=+1)
                y0, y1 = max(0, -1 - dy), min(127, 126 - dy)
                out_ap = dst[dst_row0:dst_row0 + mw,
                             y0 * W + W - 1:y1 * W + W:W]
                src_ap = pre[0:mw, (y0 + dy + 1) * W:(y1 + dy + 1) * W + 1:W]
                nc.vector.scalar_tensor_tensor(
                    out=out_ap, in0=src_ap, scalar=dwn[0:mw, ti_r:ti_r + 1],
                    in1=out_ap, op0=ALU.mult, op1=ALU.add)

        # ================= phase 1: q,k =================
        for _rep in range(reps):
         with tc.tile_pool(name="kT", bufs=1) as kTp, \
             tc.tile_pool(name="dwout", bufs=1) as dwp, \
             tc.tile_pool(name="junk", bufs=1) as jp, \
             tc.tile_pool(name="qt", bufs=3) as qtp, \
             tc.tile_pool(name="asb", bufs=1) as ap_, \
             tc.tile_pool(name="tps", bufs=2, space="PSUM") as tps, \
             tc.tile_pool(name="aps", bufs=2, space="PSUM") as aps:

            kT = kTp.tile([128, 128 * C], BF16, tag="kT")  # block b at cols b*192
            junk = jp.tile([128, 2048], BF16, tag="junk")
            stat = sp.tile([128, 8], F32, tag="stat")
            apq0 = aps.tile([128, C], F32, tag="attn")
            apq1 = aps.tile([64, C], F32, tag="attn")

            def norms_scale(ci, dw, is_q_lo, is_k_hi, q_off=None):
                """Compute 1/max(||row||,eps) (*temp for q rows), scale dw."""
                mw = CHUNKS[ci][1]
                for s4 in range(8):
                    nc.scalar.activation(
                        junk[0:mw, :], dw[0:mw, s4 * 2048:(s4 + 1) * 2048],
                        AF.Square, accum_out=stat[0:mw, s4:s4 + 1])
                n2 = sp.tile([128, 1], F32, tag="n2")
                nc.vector.tensor_reduce(n2[0:mw, :], stat[0:mw, 0:8],
                                        axis=AX.X, op=ALU.add)
                nc.scalar.activation(n2[0:mw, :], n2[0:mw, :], AF.Sqrt)
                nc.vector.tensor_scalar_max(n2[0:mw, :], n2[0:mw, :], EPS)
                rs = sp.tile([128, 1], F32, tag="rs")
                nc.vector.reciprocal(rs[0:mw, :], n2[0:mw, :])
                if is_q_lo:  # some leading rows are q channels: multiply temp
                    qw, toff = is_q_lo
                    nc.vector.tensor_tensor(
                        rs[0:qw, :], rs[0:qw, :],
                        (tmp0 if toff < 128 else tmp1)[toff % 128:toff % 128 + qw, :],
                        op=ALU.mult)
                nc.vector.tensor_scalar_mul(dw[0:mw, :], dw[0:mw, :], rs[0:mw, :])

            def transpose_blocks(ci, dw, k_cols=None, q_rows=None):
                """PE-transpose dw blocks; k cols -> kT, q cols -> attn matmuls.

                k_cols: (part_lo, part_hi, kT_off) slice of dw partitions that
                        are k channels; q_rows: (part_lo, part_hi, attn_psum,
                        attn_row0) for q channels.
                """
                mw = CHUNKS[ci][1]
                nblk = 1 if "notr" in abl else 128
                kT3 = kT[:].rearrange("p (blk c) -> p blk c", c=C)
                for b0 in range(0, nblk, 2):
                    npair = min(2, nblk - b0)
                    pt = tps.tile([128, 2 * mw], BF16, tag="t")
                    pt3 = pt[:].rearrange("p (two c) -> p two c", c=mw)
                    for h in range(npair):
                        b = b0 + h
                        nc.tensor.transpose(
                            pt[:, h * mw:(h + 1) * mw],
                            dw[0:mw, b * 128:(b + 1) * 128], eye_s[0:mw, 0:mw])
                    if k_cols is not None:
                        plo, phi, koff = k_cols
                        nc.scalar.copy(
                            kT3[:, b0:b0 + npair, koff:koff + (phi - plo)],
                            pt3[:, 0:npair, plo:phi])
                    if q_rows is not None:
                        plo, phi, apsum, arow0 = q_rows
                        qw = phi - plo
                        qtb = qtp.tile([128, 256], BF16, tag="qt")
                        nc.scalar.copy(qtb[:].rearrange(
                            "p (two c) -> p two c", c=128)[:, 0:npair, 0:qw],
                            pt3[:, 0:npair, plo:phi])
                        for h in range(npair):
                            b = b0 + h
                            nc.tensor.matmul(
                                apsum[arow0:arow0 + qw, :],
                                qtb[:, h * 128:h * 128 + qw],
                                kT[:, b * C:(b + 1) * C],
                                start=(b == 0),
                                stop=(b == (0 if "notr" in abl else 127)),
                                skip_group_check=True)

            # ---- chunk 2: k channels 256..383 (k-local 64..191) ----
            pre = prep.tile([128, N], BF16, tag="pre")
            dwd, dwn, dwpos = load_dwd(2)
            gemm_chunk(2, pre)
            dw = dwp.tile([128, N], BF16, tag="dw")
            dwconv_chunk(2, pre, dwd, dw)
            edge_fixes(2, pre, dwn, dw)
            norms_scale(2, dw, None, None)
            transpose_blocks(2, dw, k_cols=(0, 128, 64))

            # ---- chunk 1: q 128..191 (parts 0..63) + k 192..255 (parts 64..127) ----
            pre = prep.tile([128, N], BF16, tag="pre")
            dwd, dwn, dwpos = load_dwd(1)
            gemm_chunk(1, pre)
            dw = dwp.tile([128, N], BF16, tag="dw")
            dwconv_chunk(1, pre, dwd, dw)
            edge_fixes(1, pre, dwn, dw)
            norms_scale(1, dw, (64, 128), None)
            transpose_blocks(1, dw, k_cols=(64, 128, 0),
                             q_rows=(0, 64, apq1, 0))

            # ---- chunk 0: q channels 0..127 ----
            pre = prep.tile([128, N], BF16, tag="pre")
            dwd, dwn, dwpos = load_dwd(0)
            gemm_chunk(0, pre)
            dw = dwp.tile([128, N], BF16, tag="dw")
            dwconv_chunk(0, pre, dwd, dw)
            edge_fixes(0, pre, dwn, dw)
            norms_scale(0, dw, (128, 0), None)
            transpose_blocks(0, dw, q_rows=(0, 128, apq0, 0))

            # ---- attention: mask + softmax + M^T ----
            def softmax_rows(apsum, msk, mw):
                a = ap_.tile([mw, C], F32, tag=f"a{mw}")
                nc.vector.tensor_tensor(a[:], apsum[:], msk[0:mw, :], op=ALU.add)
                mx = sp.tile([128, 1], F32, tag="mx")
                nc.vector.tensor_reduce(mx[0:mw, :], a[:], axis=AX.X, op=ALU.max)
                nmx = sp.tile([128, 1], F32, tag="nmx")
                nc.vector.tensor_scalar_mul(nmx[0:mw, :], mx[0:mw, :], -1.0)
                nc.scalar.activation(a[:], a[:], AF.Exp, bias=nmx[0:mw, :])
                sm = sp.tile([128, 1], F32, tag="sm")
                nc.vector.tensor_reduce(sm[0:mw, :], a[:], axis=AX.X, op=ALU.add)
                rsm = sp.tile([128, 1], F32, tag="rsm")
                nc.vector.reciprocal(rsm[0:mw, :], sm[0:mw, :])
                nc.vector.tensor_scalar_mul(a[:], a[:], rsm[0:mw, :])
                return a
            a0 = softmax_rows(apq0, mask0, 128)
            a1 = softmax_rows(apq1, mask1, 64)

            # M^T[d,o] = sum_c A[c,d] w_outT[c,o]; K = c (192 -> 2 chunks)
            for dlo, dw_, mt in ((0, 128, mt0), (128, 64, mt1)):
                pm = tps.tile([128, C], F32, tag="t")
                nc.tensor.matmul(pm[0:dw_, :], a0[:, dlo:dlo + dw_],
                                 wo0[:], start=True, stop=False)
                nc.tensor.matmul(pm[0:dw_, :], a1[:, dlo:dlo + dw_],
                                 wo1[:], start=False, stop=True)
                nc.any.tensor_copy(mt[:], pm[0:dw_, :])

         with tc.tile_pool(name="v3", bufs=1) as v3p, \
             tc.tile_pool(name="v4", bufs=1) as v4p:
            v3 = v3p.tile([128, N], BF16, tag="v3")
            v4 = v4p.tile([64, N], BF16, tag="v4")
            for ci, vt in ((3, v3), (4, v4)):
                mw = CHUNKS[ci][1]
                pre = prep.tile([mw, N], BF16, tag="pre")
                dwd, dwn, dwpos = load_dwd(ci)
                gemm_chunk(ci, pre)
                dwconv_chunk(ci, pre, dwd, vt)
                edge_fixes(ci, pre, dwn, vt)

            for t in range(NTILES):
                cols = slice(t * NT, (t + 1) * NT)
                py0 = gps.tile([128, NT], F32, tag="g")
                nc.tensor.matmul(py0[:], mt0[:, 0:128], v3[:, cols],
                                 start=True, stop=False)
                nc.tensor.matmul(py0[:], mt1[:, 0:128], v4[:, cols],
                                 start=False, stop=True)
                y0 = yp.tile([128, NT], F32, tag="y0")
                nc.any.tensor_copy(y0[:], py0[:])
                nc.sync.dma_start(y_d[0:128, cols], y0[:])
                py1 = gps.tile([64, NT], F32, tag="g")
                nc.tensor.matmul(py1[:], mt0[:, 128:192], v3[:, cols],
                                 start=True, stop=False)
                nc.tensor.matmul(py1[:], mt1[:, 128:192], v4[:, cols],
                                 start=False, stop=True)
                y1 = yp.tile([64, NT], F32, tag="y1")
                nc.any.tensor_copy(y1[:], py1[:])
                nc.sync.dma_start(y_d[128:192, cols], y1[:])
            if yprobe_d is not None:
                nc.sync.dma_start(yprobe_d[:], y0[:, 0:4])

    nc.compile()
    return nc


def host_inputs(x, w_qkv, w_dw, w_out, temperature):
    """Host-side prep: per-core input maps."""
    b = x.shape[0]
    w_dw9 = np.asarray(w_dw, np.float32).reshape(576, 9)
    wdd = np.zeros((5, 128, 9 * 128), np.float32)
    wdn = np.zeros((5, 128, 9), np.float32)
    for ci, (s, wid) in enumerate(CHUNKS):
        for t in range(9):
            wdd[ci, :wid, t * 128:t * 128 + wid][np.arange(wid), np.arange(wid)] = \
                w_dw9[s:s + wid, t]
        wdn[ci, :wid, :] = -w_dw9[s:s + wid, :]
    temp_pc = np.repeat(np.asarray(temperature, np.float32).reshape(NHEADS), HDIM
                        ).reshape(C, 1)
    mask = np.full((C, C), -1e9, np.float32)
    for h in range(NHEADS):
        mask[h * HDIM:(h + 1) * HDIM, h * HDIM:(h + 1) * HDIM] = 0.0
    shared = {
        "w_qkvT": np.ascontiguousarray(np.asarray(w_qkv, np.float32).T
                                       ).astype(ml_dtypes.bfloat16),
        "w_dwd": wdd.astype(ml_dtypes.bfloat16),
        "w_dwn": wdn,
        "w_dwp": -wdn,
        "w_outT": np.ascontiguousarray(np.asarray(w_out, np.float32).T),
        "temp": temp_pc,
        "mask": mask,
        "eye": np.eye(128, dtype=ml_dtypes.bfloat16),
    }
    return [dict(shared, x=np.ascontiguousarray(
        np.asarray(x[c], np.float32).reshape(C, N))) for c in range(b)]


_NC_CACHE = {}


def kernel(x, w_qkv, w_dw, w_out, temperature):
    x = np.asarray(x)
    if "nc" not in _NC_CACHE:
        _NC_CACHE["nc"] = build_nc()
    nc = _NC_CACHE["nc"]
    in_maps = host_inputs(x, w_qkv, w_dw, w_out, temperature)
    res = run_bass_kernel_spmd(nc, in_maps, list(range(8)))
    out = np.stack([res.results[c]["y"].reshape(C, H, W) for c in range(8)])
    return out.astype(np.float32)
